# revision 8
# baseline (speedup 1.0000x reference)
"""Distributed multi-head attention for 8 TRN2 NeuronCores.

Problem: x[4,2048,1024], 16 heads x 64 dim, fused qkv + out proj.

Sharding: core = (batch, seq_half).  Each core computes the full
attention output for its 1024 query rows of its batch element.  K and V
are projected for the core's OWN 1024 rows only and completed by
pairwise AllGathers between the two cores of each batch pair,
overlapped with the remaining projections.  Attention is key-order
invariant, so the rank-ordered gathered buffers need no per-core fixup.

v2 changes vs the original baseline (727 us):
 - scores for the two heads of a pair run as ROW-PACKED concurrent
   matmuls (K=64 each, tile rows 0:63 / 64:127) with N=1024 bf16 PSUM
   outputs into the two banks of one [128,2048] st tile.  PE street
   time per (pair, jt) drops from ~2300 to ~1100 cycles.
 - one 2048-wide exp per (pair, jt) instead of two 1024-wide ones.
 - softmax epilogue runs on DVE+GpSimd (reciprocal + partition
   broadcast + multiply) instead of ACT copies and PE broadcasts.
 - V projection ordered before Q so the V AllGathers overlap the Q
   projection and attention starts earlier.

On-chip per core (all matmuls bf16, scores in bf16 PSUM, AV in f32):
  qT[c,i]  = wq.T @ xT         kT[c,j] = wk.T @ xT (+pair AllGather)
  V[j,c]   = xT.T @ wv (+pair AllGather), stored in head-pair blocks
             [V_even | ones-col block | V_odd] of 192 cols
  per head-pair (c-tile), per j-tile:
    st[j, 0:1024]    = kT_even^T q_even   (rows 0:63,  concurrent)
    st[j, 1024:2048] = kT_odd^T  q_odd    (rows 64:127, concurrent)
    pt = exp(0.125*st)                    (one ScalarE pass, 2048 wide)
    oacc_e += [V_e|ones]^T pt_e ; oacc_o += [ones|V_o]^T pt_o
             (PSUM accumulate over 16 j-tiles; denominators land at
              partition 64 (even) / partition 0 (odd))
  epilogue: DVE reciprocal of den row -> GpSimd partition_broadcast ->
            DVE multiply into ot (bf16)
  out[i,:] = sum_p ot_pair[p].T @ wo_p + bias   (full K=128)
"""

import numpy as np

import concourse.bass as bass
import concourse.mybir as mybir
from concourse import bacc
from concourse import bass_isa
from concourse.tile import TileContext
from concourse.bass_utils import run_bass_kernel_spmd

F32 = mybir.dt.float32
BF16 = mybir.dt.bfloat16

B, N, DIM, H, DH = 4, 2048, 1024, 16, 64
NI = N // 2  # query rows per core
SCALE = DH**-0.5
N_CORES = 8

DT = DIM // 128  # 8 contraction tiles for projections
NT = N // 128  # 16 key/value tiles
IT = NI // 128  # 8 query tiles
CT = DIM // 128  # 8 inner-dim tiles (head pairs)
# V SBUF layout per j-tile (bf16): 8 head-pair blocks of 192 cols:
#   [ V_{2p} (64) | S_p (64) | V_{2p+1} (64) ]
# where S_p is zeros with a 1.0 at its col 0.  The AV weight window for an
# even head is block cols 0:128 (V in output partitions 0:63, softmax
# denominator at partition 64); for an odd head cols 64:192 (V in
# partitions 64:127, denominator at partition 0).
VW = 192 * (H // 2)  # 1536
PAIRS = [[0, 1], [2, 3], [4, 5], [6, 7]]  # batch pairs for the K/V AllGather


def _projections(nc, tc, xT, wq, wk, wv, qT_sb, kT_sb, v_sb):
    """Q projection for the core's 1024 rows; K/V projections for the SAME
    1024 rows, then pairwise AllGathers produce the full 2048-row kT / V."""
    with (
        tc.tile_pool(name="inputs", bufs=1) as ip,
        tc.tile_pool(name="proj_psum", bufs=8, space="PSUM") as psp,
        tc.tile_pool(name="dram", bufs=1, space="DRAM") as dp,
    ):
        xT_sb = [ip.tile([128, NI], BF16, name=f"xTs{d}") for d in range(DT)]
        wq_sb = [ip.tile([128, DIM], BF16, tag=f"wqk{d}", name=f"wqs{d}") for d in range(DT)]
        wk_sb = [ip.tile([128, DIM], BF16, tag=f"wqk{d}", name=f"wks{d}") for d in range(DT)]
        wv_sb = [ip.tile([128, DIM], BF16, name=f"wvs{d}") for d in range(DT)]
        for d in range(DT):
            sl = slice(d * 128, (d + 1) * 128)
            nc.sync.dma_start(xT_sb[d][:, :], xT[sl, :])
            nc.sync.dma_start(wk_sb[d][:, :], wk[sl, :])
            nc.sync.dma_start(wv_sb[d][:, :], wv[sl, :])

        kq_stage = [ip.tile([128, NI], BF16, name=f"kq{c}") for c in range(CT)]
        # V staged directly in the 192-col pair-block layout (ones columns
        # included) so the gathered tiles read back as one contiguous DMA.
        v_stage = [ip.tile([128, VW], BF16, name=f"vs{t}") for t in range(NI // 128)]
        VG = 2  # V AllGather in 2 chunks of 4 j-tiles
        HC = CT // 2
        k_in = [dp.tile([HC * 128, NI], BF16, name=f"k_in{g}") for g in range(2)]
        k_out = [dp.tile([2 * HC * 128, NI], BF16, name=f"k_out{g}") for g in range(2)]
        v_in = [dp.tile([512, VW], BF16, name=f"v_in{g}") for g in range(VG)]
        v_out = [dp.tile([1024, VW], BF16, name=f"v_out{g}") for g in range(VG)]

        # K projection (own half) -> stage -> bounce -> 2-chunk AllGather
        for g in range(2):
            for cc in range(HC):
                c = g * HC + cc
                csl = slice(c * 128, (c + 1) * 128)
                for ch in range(NI // 512):
                    ps = psp.tile([128, 512], F32, tag="proj", name="psk")
                    jsl = slice(ch * 512, (ch + 1) * 512)
                    for d in range(DT):
                        nc.tensor.matmul(
                            ps[:, :],
                            wk_sb[d][:, csl],
                            xT_sb[d][:, jsl],
                            start=(d == 0),
                            stop=(d == DT - 1),
                        )
                    nc.vector.tensor_copy(kq_stage[c][:, jsl], ps[:, :])
                nc.sync.dma_start(k_in[g][cc * 128 : (cc + 1) * 128, :], kq_stage[c][:, :])
            nc.gpsimd.collective_compute(
                "AllGather",
                mybir.AluOpType.bypass,
                ins=[k_in[g][:, :].opt()],
                outs=[k_out[g][:, :].opt()],
                replica_groups=PAIRS,
            )
            for cc in range(HC):
                c = g * HC + cc
                hr = HC * 128
                nc.sync.dma_start(
                    kT_sb[c][:, 0:NI], k_out[g][cc * 128 : (cc + 1) * 128, :]
                )
                nc.sync.dma_start(
                    kT_sb[c][:, NI:N], k_out[g][hr + cc * 128 : hr + (cc + 1) * 128, :]
                )

        # V projection + gathers (before Q so the gathers overlap Q proj)
        for t in range(NI // 128):
            nsl = slice(t * 128, (t + 1) * 128)
            for ch in range(2):
                ps = psp.tile([128, 512], F32, tag="proj", name="psv")
                for d in range(DT):
                    nc.tensor.matmul(
                        ps[:, :],
                        xT_sb[d][:, nsl],
                        wv_sb[d][:, ch * 512 : (ch + 1) * 512],
                        start=(d == 0),
                        stop=(d == DT - 1),
                    )
                nc.vector.tensor_copy(
                    v_stage[t][:, ch * 512 : (ch + 1) * 512], ps[:, :]
                )
            g, part = t // 4, t % 4
            nc.sync.dma_start(v_in[g][part * 128 : (part + 1) * 128, :], v_stage[t][:, :])
            if part == 3:
                nc.gpsimd.collective_compute(
                    "AllGather",
                    mybir.AluOpType.bypass,
                    ins=[v_in[g][:, :].opt()],
                    outs=[v_out[g][:, :].opt()],
                    replica_groups=PAIRS,
                )
                for tt in range(NT):
                    if tt < 8:
                        gg, off = tt // 4, (tt % 4) * 128
                    else:
                        gg, off = (tt - 8) // 4, 512 + ((tt - 8) % 4) * 128
                    if gg != g:
                        continue
                    v3 = v_sb[tt][:, :].rearrange("p (a q) -> p a q", q=192)
                    nc.vector.memset(v3[:, :, 64:128], 0.0)
                    nc.vector.memset(v3[:, :, 64:65], 1.0)
                    s3 = v_out[g][off : off + 128, :].rearrange(
                        "p (a c) -> p a c", c=128
                    )
                    nc.sync.dma_start(v3[:, :, 0:DH], s3[:, :, 0:DH])
                    nc.sync.dma_start(v3[:, :, 128:192], s3[:, :, DH:128])

        for d in range(DT):
            nc.sync.dma_start(wq_sb[d][:, :], wq[d * 128 : (d + 1) * 128, :])

        # Q projection (overlaps the V collectives)
        for c in range(CT):
            csl = slice(c * 128, (c + 1) * 128)
            for ch in range(NI // 512):
                ps = psp.tile([128, 512], F32, tag="proj", name="psq")
                isl = slice(ch * 512, (ch + 1) * 512)
                for d in range(DT):
                    nc.tensor.matmul(
                        ps[:, :],
                        wq_sb[d][:, csl],
                        xT_sb[d][:, isl],
                        start=(d == 0),
                        stop=(d == DT - 1),
                    )
                nc.vector.tensor_copy(qT_sb[c][:, isl], ps[:, :])


def _av_weights(v_tile, h):
    """AV weight window for head h: 128 contiguous cols of its pair block."""
    start = 192 * (h // 2) + (64 if h % 2 else 0)
    return v_tile[:, start : start + 128]


def _attention_body(nc, psp, oap, ptp, smp, ones, qT_sb, kT_sb, v_sb, ot_sb,
                    load_wo=None):
    # Loop over head pairs (= c-tiles).  Per (pair, jt, i-half): two
    # row-packed score matmuls (K=64 each, concurrent in rows 0:63 /
    # 64:127) write one [even|odd] f32 st tile; one 1024-wide exp; AV
    # accumulates per head with the stationary kept across i-halves.
    # Each pair's epilogue is deferred behind the next pair's first
    # score matmuls; the even-head path is emitted first so the next
    # pair's first AV unblocks as early as possible.
    pending = []

    def emit_epilogue(p, oacc_e, oacc_o):
        rcp = smp.tile([128, NI], BF16, tag="rcp", name="rcp")
        nc.vector.reciprocal(rcp[64:65, :], oacc_e[64:65, :])
        nc.vector.reciprocal(rcp[0:1, :], oacc_o[0:1, :])
        rbp = psp.tile([128, NI], F32, tag="st", name="rbp")
        for half in range(2):
            fsl = slice(half * 512, (half + 1) * 512)
            nc.tensor.matmul(rbp[0:64, fsl], ones[64:65, 0:DH], rcp[64:65, fsl],
                             start=True, stop=True)
            nc.tensor.matmul(rbp[64:128, fsl], ones[0:1, 0:DH], rcp[0:1, fsl],
                             start=True, stop=True)
        rbs = smp.tile([128, NI], BF16, tag="rbs", name="rbs")
        nc.vector.tensor_copy(rbs[0:64, :], rbp[0:64, :])
        nc.vector.tensor_mul(ot_sb[p][0:64, :], oacc_e[0:64, :], rbs[0:64, :])
        nc.vector.tensor_copy(rbs[64:128, :], rbp[64:128, :])
        nc.vector.tensor_mul(ot_sb[p][64:128, :], oacc_o[64:128, :], rbs[64:128, :])

    for p in range(CT):
        if p == 2 and load_wo is not None:
            load_wo()
        oacc_e = oap.tile([128, NI], F32, tag="oacc_e", name="oacc_e")
        oacc_o = oap.tile([128, NI], F32, tag="oacc_o", name="oacc_o")
        # j-tile order matched to V AllGather chunk arrival (chunk 0 covers
        # j-tiles 0-3 and 8-11); accumulation order is irrelevant.
        jt_order = [0, 1, 2, 3, 8, 9, 10, 11, 4, 5, 6, 7, 12, 13, 14, 15]
        for ji, jt in enumerate(jt_order):
            if ji == 2 and pending:
                emit_epilogue(*pending.pop(0))
            jsl = slice(jt * 128, (jt + 1) * 128)
            # per i-half: [even 512 | odd 512] score tile; the two score
            # matmuls are row-packed (rows 0:63 / 64:127 via auto
            # tile_position) and run concurrently on the PE.
            pts = []
            for ih in range(2):
                isl = slice(ih * 512, (ih + 1) * 512)
                st = psp.tile([128, 1024], F32, tag="st", name="st")
                nc.tensor.matmul(
                    st[:, 0:512], kT_sb[p][0:64, jsl], qT_sb[p][0:64, isl],
                    start=True, stop=True,
                )
                nc.tensor.matmul(
                    st[:, 512:1024], kT_sb[p][64:128, jsl], qT_sb[p][64:128, isl],
                    start=True, stop=True,
                )
                pt = ptp.tile([128, 1024], BF16, tag="pt", name="pt")
                nc.scalar.activation(
                    pt[:, :], st[:, :], mybir.ActivationFunctionType.Exp, scale=SCALE
                )
                pts.append(pt)
            # AV: keep each head's stationary loaded across both i-halves
            for ih in range(2):
                nc.tensor.matmul(
                    oacc_e[:, ih * 512 : (ih + 1) * 512],
                    _av_weights(v_sb[jt], 2 * p),
                    pts[ih][:, 0:512],
                    start=(ji == 0),
                    stop=(ji == NT - 1),
                )
            for ih in range(2):
                nc.tensor.matmul(
                    oacc_o[:, ih * 512 : (ih + 1) * 512],
                    _av_weights(v_sb[jt], 2 * p + 1),
                    pts[ih][:, 512:1024],
                    start=(ji == 0),
                    stop=(ji == NT - 1),
                )
        pending.append((p, oacc_e, oacc_o))
    for args in pending:
        emit_epilogue(*args)


def _out_proj_body(nc, psp, outp, ot_sb, wo_sb, bias, out):
    for it in range(IT):
        itsl = slice(it * 128, (it + 1) * 128)
        psA = psp.tile([128, 512], F32, tag="opA", name="psA")
        psB = psp.tile([128, 512], F32, tag="opB", name="psB")
        for p in range(CT):
            nc.tensor.matmul(
                psA[:, :], ot_sb[p][:, itsl], wo_sb[p][:, 0:512],
                start=(p == 0), stop=(p == CT - 1),
            )
            nc.tensor.matmul(
                psB[:, :], ot_sb[p][:, itsl], wo_sb[p][:, 512:1024],
                start=(p == 0), stop=(p == CT - 1),
            )
        osb = outp.tile([128, DIM], F32, tag="osb", name="osb")
        nc.vector.tensor_add(osb[:, 0:512], psA[:, :], bias[:, 0:512])
        nc.vector.tensor_add(osb[:, 512:1024], psB[:, :], bias[:, 512:1024])
        nc.sync.dma_start(out[itsl, :], osb[:, :])


def build():
    nc = bacc.Bacc(None, target_bir_lowering=False)
    xT = nc.dram_tensor("xT", [DIM, NI], BF16, kind="ExternalInput")
    wq = nc.dram_tensor("wq", [DIM, DIM], BF16, kind="ExternalInput")
    wk = nc.dram_tensor("wk", [DIM, DIM], BF16, kind="ExternalInput")
    wv = nc.dram_tensor("wv", [DIM, DIM], BF16, kind="ExternalInput")
    wo = nc.dram_tensor("wo", [DIM, DIM], BF16, kind="ExternalInput")
    bo = nc.dram_tensor("bo", [128, DIM], F32, kind="ExternalInput")
    out = nc.dram_tensor("out", [NI, DIM], F32, kind="ExternalOutput")

    with nc.allow_low_precision("bf16 attention compute"), TileContext(nc) as tc:
        with (
            tc.tile_pool(name="persist", bufs=1) as pp,
            tc.tile_pool(name="pt_pool", bufs=4) as ptp,
            tc.tile_pool(name="small", bufs=2) as smp,
            tc.tile_pool(name="out_pool", bufs=2) as outp,
        ):
            bias = pp.tile([128, DIM], F32, name="bias")
            ones = pp.tile([128, DH], BF16, name="ones")
            nc.vector.memset(ones[:, :], 1.0)

            qT_sb = [pp.tile([128, NI], BF16, name=f"qT{c}") for c in range(CT)]
            kT_sb = [pp.tile([128, N], BF16, name=f"kT{c}") for c in range(CT)]
            v_sb = [pp.tile([128, VW], BF16, name=f"v{t}") for t in range(NT)]

            _projections(nc, tc, xT, wq, wk, wv, qT_sb, kT_sb, v_sb)

            with tc.tile_pool(name="late", bufs=1) as lp:
                ot_sb = [lp.tile([128, NI], BF16, name=f"ot{p}") for p in range(CT)]
                wo_sb = [lp.tile([128, DIM], BF16, name=f"wo{p}") for p in range(CT)]

                def load_wo():
                    nc.sync.dma_start(bias[:, :], bo[:, :])
                    for p in range(CT):
                        nc.sync.dma_start(wo_sb[p][:, :], wo[p * 128 : (p + 1) * 128, :])

                with (
                    tc.tile_pool(name="st_psum", bufs=2, space="PSUM") as psp,
                    tc.tile_pool(name="oacc_psum", bufs=1, space="PSUM") as oap,
                ):
                    _attention_body(
                        nc, psp, oap, ptp, smp, ones, qT_sb, kT_sb, v_sb, ot_sb,
                        load_wo,
                    )
                with tc.tile_pool(name="op_psum", bufs=2, space="PSUM") as opp:
                    _out_proj_body(nc, opp, outp, ot_sb, wo_sb, bias, out)

    nc.finalize()
    return nc


_CACHED_NC = None


def _get_nc():
    global _CACHED_NC
    if _CACHED_NC is None:
        _CACHED_NC = build()
    return _CACHED_NC


def _make_in_maps(x, w_qkv, w_out, b_out):
    import ml_dtypes

    bf = ml_dtypes.bfloat16
    wq = np.ascontiguousarray(w_qkv[:, 0:DIM]).astype(bf)
    wk = np.ascontiguousarray(w_qkv[:, DIM : 2 * DIM]).astype(bf)
    wv = np.ascontiguousarray(w_qkv[:, 2 * DIM : 3 * DIM]).astype(bf)
    wo = np.ascontiguousarray(w_out).astype(bf)
    bo = np.tile(np.asarray(b_out, np.float32)[None, :], (128, 1))
    in_maps = []
    for b in range(B):
        for half in range(2):
            xTh = np.ascontiguousarray(x[b, half * NI : (half + 1) * NI].T).astype(bf)
            in_maps.append(
                {"xT": xTh, "wq": wq, "wk": wk, "wv": wv, "wo": wo, "bo": bo}
            )
    return in_maps


def run_cores(in_maps, **kwargs):
    nc = _get_nc()
    return run_bass_kernel_spmd(nc, in_maps, core_ids=list(range(N_CORES)), **kwargs)


def kernel(x, mask, w_qkv, w_out, b_out):
    x = np.asarray(x, np.float32)
    res = run_cores(
        _make_in_maps(x, np.asarray(w_qkv), np.asarray(w_out), np.asarray(b_out))
    )
    out = np.empty((B, N, DIM), np.float32)
    for b in range(B):
        for half in range(2):
            out[b, half * NI : (half + 1) * NI] = res.results[b * 2 + half]["out"]
    return out


# revision 11
# speedup vs baseline: 1.1025x; 1.1025x over previous
"""Distributed multi-head attention for 8 TRN2 NeuronCores.

Problem: x[4,2048,1024], 16 heads x 64 dim, fused qkv + out proj.

Sharding: core = (batch, seq_half).  Each core computes the full
attention output for its 1024 query rows of its batch element.  K and V
are projected for the core's OWN 1024 rows only and completed by
pairwise AllGathers between the two cores of each batch pair,
overlapped with the remaining projections.  Attention is key-order
invariant, so the rank-ordered gathered buffers need no per-core fixup.

v2 changes vs the original baseline (727 us):
 - scores for the two heads of a pair run as ROW-PACKED concurrent
   matmuls (K=64 each, tile rows 0:63 / 64:127) with N=1024 bf16 PSUM
   outputs into the two banks of one [128,2048] st tile.  PE street
   time per (pair, jt) drops from ~2300 to ~1100 cycles.
 - one 2048-wide exp per (pair, jt) instead of two 1024-wide ones.
 - softmax epilogue runs on DVE+GpSimd (reciprocal + partition
   broadcast + multiply) instead of ACT copies and PE broadcasts.
 - V projection ordered before Q so the V AllGathers overlap the Q
   projection and attention starts earlier.

On-chip per core (all matmuls bf16, scores in bf16 PSUM, AV in f32):
  qT[c,i]  = wq.T @ xT         kT[c,j] = wk.T @ xT (+pair AllGather)
  V[j,c]   = xT.T @ wv (+pair AllGather), stored in head-pair blocks
             [V_even | ones-col block | V_odd] of 192 cols
  per head-pair (c-tile), per j-tile:
    st[j, 0:1024]    = kT_even^T q_even   (rows 0:63,  concurrent)
    st[j, 1024:2048] = kT_odd^T  q_odd    (rows 64:127, concurrent)
    pt = exp(0.125*st)                    (one ScalarE pass, 2048 wide)
    oacc_e += [V_e|ones]^T pt_e ; oacc_o += [ones|V_o]^T pt_o
             (PSUM accumulate over 16 j-tiles; denominators land at
              partition 64 (even) / partition 0 (odd))
  epilogue: DVE reciprocal of den row -> GpSimd partition_broadcast ->
            DVE multiply into ot (bf16)
  out[i,:] = sum_p ot_pair[p].T @ wo_p + bias   (full K=128)
"""

import numpy as np

import concourse.bass as bass
import concourse.mybir as mybir
from concourse import bacc
from concourse import bass_isa
from concourse.tile import TileContext
from concourse.bass_utils import run_bass_kernel_spmd

F32 = mybir.dt.float32
BF16 = mybir.dt.bfloat16

B, N, DIM, H, DH = 4, 2048, 1024, 16, 64
NI = N // 2  # query rows per core
SCALE = DH**-0.5
N_CORES = 8

DT = DIM // 128  # 8 contraction tiles for projections
NT = N // 128  # 16 key/value tiles
IT = NI // 128  # 8 query tiles
CT = DIM // 128  # 8 inner-dim tiles (head pairs)
# V SBUF layout per j-tile (bf16): 8 head-pair blocks of 192 cols:
#   [ V_{2p} (64) | S_p (64) | V_{2p+1} (64) ]
# where S_p is zeros with a 1.0 at its col 0.  The AV weight window for an
# even head is block cols 0:128 (V in output partitions 0:63, softmax
# denominator at partition 64); for an odd head cols 64:192 (V in
# partitions 64:127, denominator at partition 0).
VW = 192 * (H // 2)  # 1536
PAIRS = [[0, 1], [2, 3], [4, 5], [6, 7]]  # batch pairs for the K/V AllGather


def _projections(nc, tc, xT, wq, wk, wv, qT_sb, kT_sb, v_sb):
    """Q projection for the core's 1024 rows; K/V projections for the SAME
    1024 rows, then pairwise AllGathers produce the full 2048-row kT / V."""
    with (
        tc.tile_pool(name="inputs", bufs=1) as ip,
        tc.tile_pool(name="proj_psum", bufs=8, space="PSUM") as psp,
        tc.tile_pool(name="dram", bufs=1, space="DRAM") as dp,
    ):
        xT_sb = [ip.tile([128, NI], BF16, name=f"xTs{d}") for d in range(DT)]
        wq_sb = [ip.tile([128, DIM], BF16, tag=f"wqk{d}", name=f"wqs{d}") for d in range(DT)]
        wk_sb = [ip.tile([128, DIM], BF16, tag=f"wqk{d}", name=f"wks{d}") for d in range(DT)]
        wv_sb = [ip.tile([128, DIM], BF16, name=f"wvs{d}") for d in range(DT)]
        for d in range(DT):
            sl = slice(d * 128, (d + 1) * 128)
            nc.sync.dma_start(xT_sb[d][:, :], xT[sl, :])
            nc.sync.dma_start(wk_sb[d][:, :], wk[sl, :])
            nc.sync.dma_start(wv_sb[d][:, :], wv[sl, :])

        kq_stage = [ip.tile([128, NI], BF16, tag=f"kqs{c % 4}", name=f"kq{c}") for c in range(CT)]
        # V staged directly in the 192-col pair-block layout (ones columns
        # included) so the gathered tiles read back as one contiguous DMA.
        v_stage = [ip.tile([128, VW], BF16, tag=f"vstg{t % 2}", name=f"vs{t}") for t in range(NI // 128)]
        VG = 2  # V AllGather in 2 chunks of 4 j-tiles
        HC = CT // 2
        k_in = [dp.tile([HC * 128, NI], BF16, name=f"k_in{g}") for g in range(2)]
        k_out = [dp.tile([2 * HC * 128, NI], BF16, name=f"k_out{g}") for g in range(2)]
        v_in = [dp.tile([512, VW], BF16, name=f"v_in{g}") for g in range(VG)]
        v_out = [dp.tile([1024, VW], BF16, name=f"v_out{g}") for g in range(VG)]

        # K projection (own half) -> stage -> bounce -> 2-chunk AllGather
        for g in range(2):
            for cc in range(HC):
                c = g * HC + cc
                csl = slice(c * 128, (c + 1) * 128)
                for ch in range(NI // 512):
                    ps = psp.tile([128, 512], F32, tag="proj", name="psk")
                    jsl = slice(ch * 512, (ch + 1) * 512)
                    for d in range(DT):
                        nc.tensor.matmul(
                            ps[:, :],
                            wk_sb[d][:, csl],
                            xT_sb[d][:, jsl],
                            start=(d == 0),
                            stop=(d == DT - 1),
                        )
                    nc.vector.tensor_copy(kq_stage[c][:, jsl], ps[:, :])
                nc.sync.dma_start(k_in[g][cc * 128 : (cc + 1) * 128, :], kq_stage[c][:, :])
            nc.gpsimd.collective_compute(
                "AllGather",
                mybir.AluOpType.bypass,
                ins=[k_in[g][:, :].opt()],
                outs=[k_out[g][:, :].opt()],
                replica_groups=PAIRS,
            )
            for cc in range(HC):
                c = g * HC + cc
                hr = HC * 128
                nc.sync.dma_start(
                    kT_sb[c][:, 0:NI], k_out[g][cc * 128 : (cc + 1) * 128, :]
                )
                nc.sync.dma_start(
                    kT_sb[c][:, NI:N], k_out[g][hr + cc * 128 : hr + (cc + 1) * 128, :]
                )

        # V projection + gathers (before Q so the gathers overlap Q proj).
        # The stage tiles are written directly in the 192-col pair-block
        # layout (ones columns included), so the gather payload needs no
        # per-tile fixup and each gathered j-tile reads back as a single
        # contiguous DMA.
        for t in range(NI // 128):
            nsl = slice(t * 128, (t + 1) * 128)
            v3s = v_stage[t][:, :].rearrange("p (a q) -> p a q", q=192)
            nc.vector.memset(v3s[:, :, 64:128], 0.0)
            nc.vector.memset(v3s[:, :, 64:65], 1.0)
            for ch in range(2):
                ps = psp.tile([128, 512], F32, tag="proj", name="psv")
                for d in range(DT):
                    nc.tensor.matmul(
                        ps[:, :],
                        xT_sb[d][:, nsl],
                        wv_sb[d][:, ch * 512 : (ch + 1) * 512],
                        start=(d == 0),
                        stop=(d == DT - 1),
                    )
                ps4 = ps[:, :].rearrange("p (b par c) -> p b par c", par=2, c=DH)
                bsl = slice(ch * 4, (ch + 1) * 4)
                nc.vector.tensor_copy(v3s[:, bsl, 0:DH], ps4[:, :, 0, :])
                nc.vector.tensor_copy(v3s[:, bsl, 128:192], ps4[:, :, 1, :])
            g, part = t // 4, t % 4
            nc.sync.dma_start(v_in[g][part * 128 : (part + 1) * 128, :], v_stage[t][:, :])
            if part == 3:
                nc.gpsimd.collective_compute(
                    "AllGather",
                    mybir.AluOpType.bypass,
                    ins=[v_in[g][:, :].opt()],
                    outs=[v_out[g][:, :].opt()],
                    replica_groups=PAIRS,
                )
                for tt in range(NT):
                    if tt < 8:
                        gg, off = tt // 4, (tt % 4) * 128
                    else:
                        gg, off = (tt - 8) // 4, 512 + ((tt - 8) % 4) * 128
                    if gg != g:
                        continue
                    nc.sync.dma_start(v_sb[tt][:, :], v_out[g][off : off + 128, :])

        for d in range(DT):
            nc.sync.dma_start(wq_sb[d][:, :], wq[d * 128 : (d + 1) * 128, :])

        # Q projection (overlaps the V collectives)
        for c in range(CT):
            csl = slice(c * 128, (c + 1) * 128)
            for ch in range(NI // 512):
                ps = psp.tile([128, 512], F32, tag="proj", name="psq")
                isl = slice(ch * 512, (ch + 1) * 512)
                for d in range(DT):
                    nc.tensor.matmul(
                        ps[:, :],
                        wq_sb[d][:, csl],
                        xT_sb[d][:, isl],
                        start=(d == 0),
                        stop=(d == DT - 1),
                    )
                nc.vector.tensor_copy(qT_sb[c][:, isl], ps[:, :])


def _av_weights(v_tile, h):
    """AV weight window for head h: 128 contiguous cols of its pair block."""
    start = 192 * (h // 2) + (64 if h % 2 else 0)
    return v_tile[:, start : start + 128]


def _attention_body(nc, psp, oap, ptp, smp, ones, qT_sb, kT_sb, v_sb, ot_sb,
                    load_wo=None):
    # Loop over head pairs (= c-tiles).  Per (pair, jt, i-half): two
    # row-packed score matmuls (K=64 each, concurrent in rows 0:63 /
    # 64:127) write one [even|odd] f32 st tile; one 1024-wide exp; AV
    # accumulates per head with the stationary kept across i-halves.
    # Each pair's epilogue is deferred behind the next pair's first
    # score matmuls; the even-head path is emitted first so the next
    # pair's first AV unblocks as early as possible.
    pending = []

    def emit_epilogue(p, oacc_e, oacc_o):
        # Copy the PSUM accumulators to SBUF first: this is the ONLY step
        # the next pair's AV matmuls wait on (it frees the oacc banks in
        # ~1.2us); the reciprocal/broadcast/multiply chain then runs off
        # the critical path from the SBUF copies.
        oaS_e = smp.tile([128, NI], F32, tag="oaS_e", name="oaS_e")
        oaS_o = smp.tile([128, NI], F32, tag="oaS_o", name="oaS_o")
        nc.vector.tensor_copy(oaS_e[:, :], oacc_e[:, :])
        nc.vector.tensor_copy(oaS_o[:, :], oacc_o[:, :])
        rcp = smp.tile([128, NI], F32, tag="rcp", name="rcp")
        nc.vector.reciprocal_approx_fast(rcp[64:65, :], oaS_e[64:65, :])
        nc.vector.reciprocal_approx_fast(rcp[0:1, :], oaS_o[0:1, :])
        rcpb = smp.tile([128, NI], BF16, tag="rcpb", name="rcpb")
        nc.vector.tensor_copy(rcpb[64:65, :], rcp[64:65, :])
        nc.vector.tensor_copy(rcpb[0:1, :], rcp[0:1, :])
        rbp = psp.tile([128, NI], F32, tag="st", name="rbp")
        for half in range(2):
            fsl = slice(half * 512, (half + 1) * 512)
            nc.tensor.matmul(rbp[0:64, fsl], ones[64:65, 0:DH], rcpb[64:65, fsl],
                             start=True, stop=True)
            nc.tensor.matmul(rbp[64:128, fsl], ones[0:1, 0:DH], rcpb[0:1, fsl],
                             start=True, stop=True)
        nc.vector.tensor_mul(ot_sb[p][0:64, :], oaS_e[0:64, :], rbp[0:64, :])
        nc.vector.tensor_mul(ot_sb[p][64:128, :], oaS_o[64:128, :], rbp[64:128, :])

    for p in range(CT):
        if p == 2 and load_wo is not None:
            load_wo()
        oacc_e = oap.tile([128, NI], F32, tag="oacc_e", name="oacc_e")
        oacc_o = oap.tile([128, NI], F32, tag="oacc_o", name="oacc_o")
        # j-tile order matched to V AllGather chunk arrival (chunk 0 covers
        # j-tiles 0-3 and 8-11); accumulation order is irrelevant.
        jt_order = [0, 1, 2, 3, 8, 9, 10, 11, 4, 5, 6, 7, 12, 13, 14, 15]
        for ji, jt in enumerate(jt_order):
            if ji == 2 and pending:
                emit_epilogue(*pending.pop(0))
            jsl = slice(jt * 128, (jt + 1) * 128)
            # per i-half: [even 512 | odd 512] score tile; the two score
            # matmuls are row-packed (rows 0:63 / 64:127 via auto
            # tile_position) and run concurrently on the PE.
            pts = []
            for ih in range(2):
                isl = slice(ih * 512, (ih + 1) * 512)
                st = psp.tile([128, 1024], F32, tag="st", name="st")
                nc.tensor.matmul(
                    st[:, 0:512], kT_sb[p][0:64, jsl], qT_sb[p][0:64, isl],
                    start=True, stop=True,
                )
                nc.tensor.matmul(
                    st[:, 512:1024], kT_sb[p][64:128, jsl], qT_sb[p][64:128, isl],
                    start=True, stop=True,
                )
                pt = ptp.tile([128, 1024], BF16, tag="pt", name="pt")
                nc.scalar.activation(
                    pt[:, :], st[:, :], mybir.ActivationFunctionType.Exp, scale=SCALE
                )
                pts.append(pt)
            # AV: keep each head's stationary loaded across both i-halves
            for ih in range(2):
                nc.tensor.matmul(
                    oacc_e[:, ih * 512 : (ih + 1) * 512],
                    _av_weights(v_sb[jt], 2 * p),
                    pts[ih][:, 0:512],
                    start=(ji == 0),
                    stop=(ji == NT - 1),
                )
            for ih in range(2):
                nc.tensor.matmul(
                    oacc_o[:, ih * 512 : (ih + 1) * 512],
                    _av_weights(v_sb[jt], 2 * p + 1),
                    pts[ih][:, 512:1024],
                    start=(ji == 0),
                    stop=(ji == NT - 1),
                )
        pending.append((p, oacc_e, oacc_o))
    for args in pending:
        emit_epilogue(*args)


def _out_proj_body(nc, psp, outp, ot_sb, wo_sb, bias, out):
    for it in range(IT):
        itsl = slice(it * 128, (it + 1) * 128)
        psA = psp.tile([128, 512], F32, tag="opA", name="psA")
        psB = psp.tile([128, 512], F32, tag="opB", name="psB")
        for p in range(CT):
            nc.tensor.matmul(
                psA[:, :], ot_sb[p][:, itsl], wo_sb[p][:, 0:512],
                start=(p == 0), stop=(p == CT - 1),
            )
            nc.tensor.matmul(
                psB[:, :], ot_sb[p][:, itsl], wo_sb[p][:, 512:1024],
                start=(p == 0), stop=(p == CT - 1),
            )
        osb = outp.tile([128, DIM], F32, tag="osb", name="osb")
        nc.vector.tensor_add(osb[:, 0:512], psA[:, :], bias[:, 0:512])
        nc.vector.tensor_add(osb[:, 512:1024], psB[:, :], bias[:, 512:1024])
        nc.sync.dma_start(out[itsl, :], osb[:, :])


def build():
    nc = bacc.Bacc(None, target_bir_lowering=False)
    xT = nc.dram_tensor("xT", [DIM, NI], BF16, kind="ExternalInput")
    wq = nc.dram_tensor("wq", [DIM, DIM], BF16, kind="ExternalInput")
    wk = nc.dram_tensor("wk", [DIM, DIM], BF16, kind="ExternalInput")
    wv = nc.dram_tensor("wv", [DIM, DIM], BF16, kind="ExternalInput")
    wo = nc.dram_tensor("wo", [DIM, DIM], BF16, kind="ExternalInput")
    bo = nc.dram_tensor("bo", [128, DIM], F32, kind="ExternalInput")
    out = nc.dram_tensor("out", [NI, DIM], F32, kind="ExternalOutput")

    with nc.allow_low_precision("bf16 attention compute"), TileContext(nc) as tc:
        with (
            tc.tile_pool(name="persist", bufs=1) as pp,
            tc.tile_pool(name="pt_pool", bufs=4) as ptp,
        ):
            bias = pp.tile([128, DIM], F32, name="bias")
            ones = pp.tile([128, DH], BF16, name="ones")
            nc.vector.memset(ones[:, :], 1.0)

            qT_sb = [pp.tile([128, NI], BF16, name=f"qT{c}") for c in range(CT)]
            kT_sb = [pp.tile([128, N], BF16, name=f"kT{c}") for c in range(CT)]
            v_sb = [pp.tile([128, VW], BF16, name=f"v{t}") for t in range(NT)]

            _projections(nc, tc, xT, wq, wk, wv, qT_sb, kT_sb, v_sb)

            with (
                tc.tile_pool(name="late", bufs=1) as lp,
                tc.tile_pool(name="small", bufs=1) as smp,
                tc.tile_pool(name="out_pool", bufs=2) as outp,
            ):
                ot_sb = [lp.tile([128, NI], BF16, name=f"ot{p}") for p in range(CT)]
                wo_sb = [lp.tile([128, DIM], BF16, name=f"wo{p}") for p in range(CT)]

                def load_wo():
                    nc.sync.dma_start(bias[:, :], bo[:, :])
                    for p in range(CT):
                        nc.sync.dma_start(wo_sb[p][:, :], wo[p * 128 : (p + 1) * 128, :])

                with (
                    tc.tile_pool(name="st_psum", bufs=2, space="PSUM") as psp,
                    tc.tile_pool(name="oacc_psum", bufs=1, space="PSUM") as oap,
                ):
                    _attention_body(
                        nc, psp, oap, ptp, smp, ones, qT_sb, kT_sb, v_sb, ot_sb,
                        load_wo,
                    )
                with tc.tile_pool(name="op_psum", bufs=2, space="PSUM") as opp:
                    _out_proj_body(nc, opp, outp, ot_sb, wo_sb, bias, out)

    nc.finalize()
    return nc


_CACHED_NC = None


def _get_nc():
    global _CACHED_NC
    if _CACHED_NC is None:
        _CACHED_NC = build()
    return _CACHED_NC


def _make_in_maps(x, w_qkv, w_out, b_out):
    import ml_dtypes

    bf = ml_dtypes.bfloat16
    wq = np.ascontiguousarray(w_qkv[:, 0:DIM]).astype(bf)
    wk = np.ascontiguousarray(w_qkv[:, DIM : 2 * DIM]).astype(bf)
    wv = np.ascontiguousarray(w_qkv[:, 2 * DIM : 3 * DIM]).astype(bf)
    wo = np.ascontiguousarray(w_out).astype(bf)
    bo = np.tile(np.asarray(b_out, np.float32)[None, :], (128, 1))
    in_maps = []
    for b in range(B):
        for half in range(2):
            xTh = np.ascontiguousarray(x[b, half * NI : (half + 1) * NI].T).astype(bf)
            in_maps.append(
                {"xT": xTh, "wq": wq, "wk": wk, "wv": wv, "wo": wo, "bo": bo}
            )
    return in_maps


def run_cores(in_maps, **kwargs):
    nc = _get_nc()
    return run_bass_kernel_spmd(nc, in_maps, core_ids=list(range(N_CORES)), **kwargs)


def kernel(x, mask, w_qkv, w_out, b_out):
    x = np.asarray(x, np.float32)
    res = run_cores(
        _make_in_maps(x, np.asarray(w_qkv), np.asarray(w_out), np.asarray(b_out))
    )
    out = np.empty((B, N, DIM), np.float32)
    for b in range(B):
        for half in range(2):
            out[b, half * NI : (half + 1) * NI] = res.results[b * 2 + half]["out"]
    return out


# revision 12
# speedup vs baseline: 1.1334x; 1.0281x over previous
"""Distributed multi-head attention for 8 TRN2 NeuronCores.

Problem: x[4,2048,1024], 16 heads x 64 dim, fused qkv + out proj.

Sharding: core = (batch, seq_half).  Each core computes the full
attention output for its 1024 query rows of its batch element.  K and V
are projected for the core's OWN 1024 rows only and completed by
pairwise AllGathers between the two cores of each batch pair,
overlapped with the remaining projections.  Attention is key-order
invariant, so the rank-ordered gathered buffers need no per-core fixup.

v2 changes vs the original baseline (727 us):
 - scores for the two heads of a pair run as ROW-PACKED concurrent
   matmuls (K=64 each, tile rows 0:63 / 64:127) with N=1024 bf16 PSUM
   outputs into the two banks of one [128,2048] st tile.  PE street
   time per (pair, jt) drops from ~2300 to ~1100 cycles.
 - one 2048-wide exp per (pair, jt) instead of two 1024-wide ones.
 - softmax epilogue runs on DVE+GpSimd (reciprocal + partition
   broadcast + multiply) instead of ACT copies and PE broadcasts.
 - V projection ordered before Q so the V AllGathers overlap the Q
   projection and attention starts earlier.

On-chip per core (all matmuls bf16, scores in bf16 PSUM, AV in f32):
  qT[c,i]  = wq.T @ xT         kT[c,j] = wk.T @ xT (+pair AllGather)
  V[j,c]   = xT.T @ wv (+pair AllGather), stored in head-pair blocks
             [V_even | ones-col block | V_odd] of 192 cols
  per head-pair (c-tile), per j-tile:
    st[j, 0:1024]    = kT_even^T q_even   (rows 0:63,  concurrent)
    st[j, 1024:2048] = kT_odd^T  q_odd    (rows 64:127, concurrent)
    pt = exp(0.125*st)                    (one ScalarE pass, 2048 wide)
    oacc_e += [V_e|ones]^T pt_e ; oacc_o += [ones|V_o]^T pt_o
             (PSUM accumulate over 16 j-tiles; denominators land at
              partition 64 (even) / partition 0 (odd))
  epilogue: DVE reciprocal of den row -> GpSimd partition_broadcast ->
            DVE multiply into ot (bf16)
  out[i,:] = sum_p ot_pair[p].T @ wo_p + bias   (full K=128)
"""

import numpy as np

import concourse.bass as bass
import concourse.mybir as mybir
from concourse import bacc
from concourse import bass_isa
from concourse.tile import TileContext
from concourse.bass_utils import run_bass_kernel_spmd

F32 = mybir.dt.float32
BF16 = mybir.dt.bfloat16

B, N, DIM, H, DH = 4, 2048, 1024, 16, 64
NI = N // 2  # query rows per core
SCALE = DH**-0.5
N_CORES = 8

DT = DIM // 128  # 8 contraction tiles for projections
NT = N // 128  # 16 key/value tiles
IT = NI // 128  # 8 query tiles
CT = DIM // 128  # 8 inner-dim tiles (head pairs)
# V SBUF layout per j-tile (bf16): 8 head-pair blocks of 192 cols:
#   [ V_{2p} (64) | S_p (64) | V_{2p+1} (64) ]
# where S_p is zeros with a 1.0 at its col 0.  The AV weight window for an
# even head is block cols 0:128 (V in output partitions 0:63, softmax
# denominator at partition 64); for an odd head cols 64:192 (V in
# partitions 64:127, denominator at partition 0).
VW = 192 * (H // 2)  # 1536
PAIRS = [[0, 1], [2, 3], [4, 5], [6, 7]]  # batch pairs for the K/V AllGather


def _projections(nc, tc, xT, wq, wk, wv, qT_sb, kT_sb, v_sb):
    """Q projection for the core's 1024 rows; K/V projections for the SAME
    1024 rows, then pairwise AllGathers produce the full 2048-row kT / V."""
    with (
        tc.tile_pool(name="inputs", bufs=1) as ip,
        tc.tile_pool(name="proj_psum", bufs=8, space="PSUM") as psp,
        tc.tile_pool(name="dram", bufs=1, space="DRAM") as dp,
    ):
        xT_sb = [ip.tile([128, NI], BF16, name=f"xTs{d}") for d in range(DT)]
        wq_sb = [ip.tile([128, DIM], BF16, tag=f"wqk{d}", name=f"wqs{d}") for d in range(DT)]
        wk_sb = [ip.tile([128, DIM], BF16, tag=f"wqk{d}", name=f"wks{d}") for d in range(DT)]
        wv_sb = [ip.tile([128, DIM], BF16, name=f"wvs{d}") for d in range(DT)]
        for d in range(DT):
            sl = slice(d * 128, (d + 1) * 128)
            nc.sync.dma_start(xT_sb[d][:, :], xT[sl, :])
            nc.sync.dma_start(wk_sb[d][:, :], wk[sl, :])
            nc.sync.dma_start(wv_sb[d][:, :], wv[sl, :])

        kq_stage = [ip.tile([128, NI], BF16, tag=f"kqs{c % 4}", name=f"kq{c}") for c in range(CT)]
        # V staged directly in the 192-col pair-block layout (ones columns
        # included) so the gathered tiles read back as one contiguous DMA.
        v_stage = [ip.tile([128, VW], BF16, tag=f"vstg{t % 2}", name=f"vs{t}") for t in range(NI // 128)]
        VG = 2  # V AllGather in 2 chunks of 4 j-tiles
        HC = CT // 2
        k_in = [dp.tile([HC * 128, NI], BF16, name=f"k_in{g}") for g in range(2)]
        k_out = [dp.tile([2 * HC * 128, NI], BF16, name=f"k_out{g}") for g in range(2)]
        v_in = [dp.tile([512, VW], BF16, name=f"v_in{g}") for g in range(VG)]
        v_out = [dp.tile([1024, VW], BF16, name=f"v_out{g}") for g in range(VG)]

        # K projection (own half) -> stage -> bounce -> 2-chunk AllGather
        for g in range(2):
            for cc in range(HC):
                c = g * HC + cc
                csl = slice(c * 128, (c + 1) * 128)
                for ch in range(NI // 512):
                    ps = psp.tile([128, 512], F32, tag="proj", name="psk")
                    jsl = slice(ch * 512, (ch + 1) * 512)
                    for d in range(DT):
                        nc.tensor.matmul(
                            ps[:, :],
                            wk_sb[d][:, csl],
                            xT_sb[d][:, jsl],
                            start=(d == 0),
                            stop=(d == DT - 1),
                        )
                    nc.vector.tensor_copy(kq_stage[c][:, jsl], ps[:, :])
                nc.sync.dma_start(k_in[g][cc * 128 : (cc + 1) * 128, :], kq_stage[c][:, :])
            nc.gpsimd.collective_compute(
                "AllGather",
                mybir.AluOpType.bypass,
                ins=[k_in[g][:, :].opt()],
                outs=[k_out[g][:, :].opt()],
                replica_groups=PAIRS,
            )
            for cc in range(HC):
                c = g * HC + cc
                hr = HC * 128
                nc.sync.dma_start(
                    kT_sb[c][:, 0:NI], k_out[g][cc * 128 : (cc + 1) * 128, :]
                )
                nc.sync.dma_start(
                    kT_sb[c][:, NI:N], k_out[g][hr + cc * 128 : hr + (cc + 1) * 128, :]
                )

        # V projection + gathers (before Q so the gathers overlap Q proj).
        # The stage tiles are written directly in the 192-col pair-block
        # layout (ones columns included), so the gather payload needs no
        # per-tile fixup and each gathered j-tile reads back as a single
        # contiguous DMA.
        for t in range(NI // 128):
            nsl = slice(t * 128, (t + 1) * 128)
            v3s = v_stage[t][:, :].rearrange("p (a q) -> p a q", q=192)
            nc.vector.memset(v3s[:, :, 64:128], 0.0)
            nc.vector.memset(v3s[:, :, 64:65], 1.0)
            for ch in range(2):
                ps = psp.tile([128, 512], F32, tag="proj", name="psv")
                for d in range(DT):
                    nc.tensor.matmul(
                        ps[:, :],
                        xT_sb[d][:, nsl],
                        wv_sb[d][:, ch * 512 : (ch + 1) * 512],
                        start=(d == 0),
                        stop=(d == DT - 1),
                    )
                ps4 = ps[:, :].rearrange("p (b par c) -> p b par c", par=2, c=DH)
                bsl = slice(ch * 4, (ch + 1) * 4)
                nc.vector.tensor_copy(v3s[:, bsl, 0:DH], ps4[:, :, 0, :])
                nc.vector.tensor_copy(v3s[:, bsl, 128:192], ps4[:, :, 1, :])
            g, part = t // 4, t % 4
            nc.sync.dma_start(v_in[g][part * 128 : (part + 1) * 128, :], v_stage[t][:, :])
            if part == 3:
                nc.gpsimd.collective_compute(
                    "AllGather",
                    mybir.AluOpType.bypass,
                    ins=[v_in[g][:, :].opt()],
                    outs=[v_out[g][:, :].opt()],
                    replica_groups=PAIRS,
                )
                for tt in range(NT):
                    if tt < 8:
                        gg, off = tt // 4, (tt % 4) * 128
                    else:
                        gg, off = (tt - 8) // 4, 512 + ((tt - 8) % 4) * 128
                    if gg != g:
                        continue
                    nc.sync.dma_start(v_sb[tt][:, :], v_out[g][off : off + 128, :])

        for d in range(DT):
            nc.sync.dma_start(wq_sb[d][:, :], wq[d * 128 : (d + 1) * 128, :])

        # Q projection (overlaps the V collectives)
        for c in range(CT):
            csl = slice(c * 128, (c + 1) * 128)
            for ch in range(NI // 512):
                ps = psp.tile([128, 512], F32, tag="proj", name="psq")
                isl = slice(ch * 512, (ch + 1) * 512)
                for d in range(DT):
                    nc.tensor.matmul(
                        ps[:, :],
                        wq_sb[d][:, csl],
                        xT_sb[d][:, isl],
                        start=(d == 0),
                        stop=(d == DT - 1),
                    )
                nc.vector.tensor_copy(qT_sb[c][:, isl], ps[:, :])


def _av_weights(v_tile, h):
    """AV weight window for head h: 128 contiguous cols of its pair block."""
    start = 192 * (h // 2) + (64 if h % 2 else 0)
    return v_tile[:, start : start + 128]


def _attention_body(nc, psp, oap, ptp, smp, ones, qT_sb, kT_sb, v_sb, ot_sb,
                    load_wo=None):
    # Loop over head pairs (= c-tiles).  Per (pair, jt, i-half): two
    # row-packed score matmuls (K=64 each, concurrent in rows 0:63 /
    # 64:127) write one [even|odd] f32 st tile; one 1024-wide exp; AV
    # accumulates per head with the stationary kept across i-halves.
    # Each pair's epilogue is deferred behind the next pair's first
    # score matmuls; the even-head path is emitted first so the next
    # pair's first AV unblocks as early as possible.
    pending = []

    def emit_epilogue(p, oacc_e, oacc_o):
        # Copy the PSUM accumulators to SBUF first: this is the ONLY step
        # the next pair's AV matmuls wait on (it frees the oacc banks in
        # ~1.2us); the reciprocal/broadcast/multiply chain then runs off
        # the critical path from the SBUF copies.
        oaS_e = smp.tile([128, NI], F32, tag="oaS_e", name="oaS_e")
        oaS_o = smp.tile([128, NI], F32, tag="oaS_o", name="oaS_o")
        nc.vector.tensor_copy(oaS_e[:, :], oacc_e[:, :])
        nc.vector.tensor_copy(oaS_o[:, :], oacc_o[:, :])
        # both denominators on partition 0 (reciprocal_approx_fast mishandles
        # nonzero base partitions on HW): DMA moves the even den row from
        # partition 64; the odd den row is already on partition 0.
        den2 = smp.tile([128, 2 * NI], F32, tag="den2", name="den2")
        nc.sync.dma_start(den2[0:1, 0:NI], oaS_e[64:65, :])
        nc.vector.tensor_copy(den2[0:1, NI : 2 * NI], oaS_o[0:1, :])
        rcp = smp.tile([128, 2 * NI], F32, tag="rcp", name="rcp")
        nc.vector.reciprocal_approx_fast(rcp[0:1, :], den2[0:1, :])
        rcpb = smp.tile([128, 2 * NI], BF16, tag="rcpb", name="rcpb")
        nc.vector.tensor_copy(rcpb[0:1, :], rcp[0:1, :])
        rbp = psp.tile([128, NI], F32, tag="st", name="rbp")
        for half in range(2):
            fsl = slice(half * 512, (half + 1) * 512)
            osl = slice(NI + half * 512, NI + (half + 1) * 512)
            nc.tensor.matmul(rbp[0:64, fsl], ones[0:1, 0:DH], rcpb[0:1, fsl],
                             start=True, stop=True)
            nc.tensor.matmul(rbp[64:128, fsl], ones[0:1, 0:DH], rcpb[0:1, osl],
                             start=True, stop=True)
        nc.vector.tensor_mul(ot_sb[p][0:64, :], oaS_e[0:64, :], rbp[0:64, :])
        nc.vector.tensor_mul(ot_sb[p][64:128, :], oaS_o[64:128, :], rbp[64:128, :])

    for p in range(CT):
        if p == 2 and load_wo is not None:
            load_wo()
        oacc_e = oap.tile([128, NI], F32, tag="oacc_e", name="oacc_e")
        oacc_o = oap.tile([128, NI], F32, tag="oacc_o", name="oacc_o")
        # j-tile order matched to V AllGather chunk arrival (chunk 0 covers
        # j-tiles 0-3 and 8-11); accumulation order is irrelevant.
        jt_order = [0, 1, 2, 3, 8, 9, 10, 11, 4, 5, 6, 7, 12, 13, 14, 15]
        for ji, jt in enumerate(jt_order):
            if ji == 2 and pending:
                emit_epilogue(*pending.pop(0))
            jsl = slice(jt * 128, (jt + 1) * 128)
            # per i-half: [even 512 | odd 512] score tile; the two score
            # matmuls are row-packed (rows 0:63 / 64:127 via auto
            # tile_position) and run concurrently on the PE.
            pts = []
            for ih in range(2):
                isl = slice(ih * 512, (ih + 1) * 512)
                st = psp.tile([128, 1024], F32, tag="st", name="st")
                nc.tensor.matmul(
                    st[:, 0:512], kT_sb[p][0:64, jsl], qT_sb[p][0:64, isl],
                    start=True, stop=True,
                )
                nc.tensor.matmul(
                    st[:, 512:1024], kT_sb[p][64:128, jsl], qT_sb[p][64:128, isl],
                    start=True, stop=True,
                )
                pt = ptp.tile([128, 1024], BF16, tag="pt", name="pt")
                nc.scalar.activation(
                    pt[:, :], st[:, :], mybir.ActivationFunctionType.Exp, scale=SCALE
                )
                pts.append(pt)
            # AV: keep each head's stationary loaded across both i-halves
            for ih in range(2):
                nc.tensor.matmul(
                    oacc_e[:, ih * 512 : (ih + 1) * 512],
                    _av_weights(v_sb[jt], 2 * p),
                    pts[ih][:, 0:512],
                    start=(ji == 0),
                    stop=(ji == NT - 1),
                )
            for ih in range(2):
                nc.tensor.matmul(
                    oacc_o[:, ih * 512 : (ih + 1) * 512],
                    _av_weights(v_sb[jt], 2 * p + 1),
                    pts[ih][:, 512:1024],
                    start=(ji == 0),
                    stop=(ji == NT - 1),
                )
        pending.append((p, oacc_e, oacc_o))
    for args in pending:
        emit_epilogue(*args)


def _out_proj_body(nc, psp, outp, ot_sb, wo_sb, bias, out):
    for it in range(IT):
        itsl = slice(it * 128, (it + 1) * 128)
        psA = psp.tile([128, 512], F32, tag="opA", name="psA")
        psB = psp.tile([128, 512], F32, tag="opB", name="psB")
        for p in range(CT):
            nc.tensor.matmul(
                psA[:, :], ot_sb[p][:, itsl], wo_sb[p][:, 0:512],
                start=(p == 0), stop=(p == CT - 1),
            )
            nc.tensor.matmul(
                psB[:, :], ot_sb[p][:, itsl], wo_sb[p][:, 512:1024],
                start=(p == 0), stop=(p == CT - 1),
            )
        osb = outp.tile([128, DIM], F32, tag="osb", name="osb")
        nc.vector.tensor_add(osb[:, 0:512], psA[:, :], bias[:, 0:512])
        nc.vector.tensor_add(osb[:, 512:1024], psB[:, :], bias[:, 512:1024])
        nc.sync.dma_start(out[itsl, :], osb[:, :])


def build():
    nc = bacc.Bacc(None, target_bir_lowering=False)
    xT = nc.dram_tensor("xT", [DIM, NI], BF16, kind="ExternalInput")
    wq = nc.dram_tensor("wq", [DIM, DIM], BF16, kind="ExternalInput")
    wk = nc.dram_tensor("wk", [DIM, DIM], BF16, kind="ExternalInput")
    wv = nc.dram_tensor("wv", [DIM, DIM], BF16, kind="ExternalInput")
    wo = nc.dram_tensor("wo", [DIM, DIM], BF16, kind="ExternalInput")
    bo = nc.dram_tensor("bo", [128, DIM], F32, kind="ExternalInput")
    out = nc.dram_tensor("out", [NI, DIM], F32, kind="ExternalOutput")

    with nc.allow_low_precision("bf16 attention compute"), TileContext(nc) as tc:
        with (
            tc.tile_pool(name="persist", bufs=1) as pp,
            tc.tile_pool(name="pt_pool", bufs=4) as ptp,
        ):
            bias = pp.tile([128, DIM], F32, name="bias")
            ones = pp.tile([128, DH], BF16, name="ones")
            nc.vector.memset(ones[:, :], 1.0)

            qT_sb = [pp.tile([128, NI], BF16, name=f"qT{c}") for c in range(CT)]
            kT_sb = [pp.tile([128, N], BF16, name=f"kT{c}") for c in range(CT)]
            v_sb = [pp.tile([128, VW], BF16, name=f"v{t}") for t in range(NT)]

            _projections(nc, tc, xT, wq, wk, wv, qT_sb, kT_sb, v_sb)

            with (
                tc.tile_pool(name="late", bufs=1) as lp,
                tc.tile_pool(name="small", bufs=1) as smp,
                tc.tile_pool(name="out_pool", bufs=2) as outp,
            ):
                ot_sb = [lp.tile([128, NI], BF16, name=f"ot{p}") for p in range(CT)]
                wo_sb = [lp.tile([128, DIM], BF16, name=f"wo{p}") for p in range(CT)]

                def load_wo():
                    nc.sync.dma_start(bias[:, :], bo[:, :])
                    for p in range(CT):
                        nc.sync.dma_start(wo_sb[p][:, :], wo[p * 128 : (p + 1) * 128, :])

                with (
                    tc.tile_pool(name="st_psum", bufs=2, space="PSUM") as psp,
                    tc.tile_pool(name="oacc_psum", bufs=1, space="PSUM") as oap,
                ):
                    _attention_body(
                        nc, psp, oap, ptp, smp, ones, qT_sb, kT_sb, v_sb, ot_sb,
                        load_wo,
                    )
                with tc.tile_pool(name="op_psum", bufs=2, space="PSUM") as opp:
                    _out_proj_body(nc, opp, outp, ot_sb, wo_sb, bias, out)

    nc.finalize()
    return nc


_CACHED_NC = None


def _get_nc():
    global _CACHED_NC
    if _CACHED_NC is None:
        _CACHED_NC = build()
    return _CACHED_NC


def _make_in_maps(x, w_qkv, w_out, b_out):
    import ml_dtypes

    bf = ml_dtypes.bfloat16
    wq = np.ascontiguousarray(w_qkv[:, 0:DIM]).astype(bf)
    wk = np.ascontiguousarray(w_qkv[:, DIM : 2 * DIM]).astype(bf)
    wv = np.ascontiguousarray(w_qkv[:, 2 * DIM : 3 * DIM]).astype(bf)
    wo = np.ascontiguousarray(w_out).astype(bf)
    bo = np.tile(np.asarray(b_out, np.float32)[None, :], (128, 1))
    in_maps = []
    for b in range(B):
        for half in range(2):
            xTh = np.ascontiguousarray(x[b, half * NI : (half + 1) * NI].T).astype(bf)
            in_maps.append(
                {"xT": xTh, "wq": wq, "wk": wk, "wv": wv, "wo": wo, "bo": bo}
            )
    return in_maps


def run_cores(in_maps, **kwargs):
    nc = _get_nc()
    return run_bass_kernel_spmd(nc, in_maps, core_ids=list(range(N_CORES)), **kwargs)


def kernel(x, mask, w_qkv, w_out, b_out):
    x = np.asarray(x, np.float32)
    res = run_cores(
        _make_in_maps(x, np.asarray(w_qkv), np.asarray(w_out), np.asarray(b_out))
    )
    out = np.empty((B, N, DIM), np.float32)
    for b in range(B):
        for half in range(2):
            out[b, half * NI : (half + 1) * NI] = res.results[b * 2 + half]["out"]
    return out
